# revision 33
# baseline (speedup 1.0000x reference)
"""Bass/Trainium2 kernel for the EvolutionAgentQuantum circuit.

10-qubit state-vector simulation, batch 4096, 5 layers of
[per-sample RY] -> [fused RZ diagonal] -> [shared RY + CNOT ring],
then 4 Pauli-Z expectations. Data-parallel over 8 NeuronCores
(512 samples each), 4 partition-chains of 128 samples per core.

Layout per chain: state re/im in one SBUF tile [128, 2048] fp16
(free = comp*1024 + amplitude; qubit q has amplitude stride 2^(9-q)).

Key speed tricks vs the fp32 version:
 - state, tan-butterflies, diag, and matmuls all fp16: the DVE
   TensorScalarPtr butterflies hit the 4x_2p perf mode, PE matmuls
   and transposes run at 1 cycle/row.
 - per-layer phase table built by a tiny PE matmul against a +-1
   sign matrix (fp16 hi/lo split of the angles for fp32-level
   accuracy) instead of a ScalarE doubling recursion.
 - sample-major -> amp-major transposes done by the DMA XBAR
   (dma_start(transpose=True)) straight into fp16 lhsT blocks: no
   PE transposes, no PSUM staging copies.
 - range reduction in "turns" units: one magic-round Activation op
   plus one fused STT; sin via Act Sin with scale folding, cos via
   cos(x) = sin(pi/2 - |x|) (Abs + Sin, always inside the table range).
 - engine placement tuned to measured HW throughputs: DVE does the
   butterfly mults/adds (fp16 2x/4x modes), Act takes some butterfly
   products + all activations, Pool only fp32/fp16 side products.
"""

import sys
import os

for _p in ("/opt/trn_rl_repo", "/root/.axon_site/_ro/trn_rl_repo"):
    if os.path.isdir(_p) and _p not in sys.path:
        sys.path.insert(0, _p)

import numpy as np

import concourse.bacc as bacc
from concourse import mybir
from concourse.bass_utils import run_bass_kernel_spmd
from concourse.tile import TileContext

AF = mybir.ActivationFunctionType
ALU = mybir.AluOpType
F32 = mybir.dt.float32
F16 = mybir.dt.float16

N_CORES = 8
BATCH = 4096
OBS = 10
NQ = 10
NL = 5
NOUT = 4
PER_CORE = BATCH // N_CORES      # 512
P = 128
NCHAIN = PER_CORE // P           # 4
NAMP = 1 << NQ                   # 1024
SFREE = 2 * NAMP                 # 2048 (re | im)

PI = float(np.pi)
HALF_PI = float(np.pi / 2)
TWO_PI = float(2 * np.pi)
MAGIC = float(1.5 * 2 ** 23)

# host table layout (columns of the broadcast [128, TBL_LEN] tile)
O_YS = 0      # 50: theta_y/2 scale (0.5*isc[l, i]), radians
O_ZS = 50     # 50: phi_x scale in turns (0.5*isc[l, i+10] / 2pi)
O_ZW = 100    # 50: phi_w additive in turns (0.5*w[l, i] / 2pi)
O_OS = 150    # 4:  output_scaling
TBL_LEN = 154

_CACHED_NC = {}


def _emit_butterfly_stt(eng, src, dst, q, tcol, ntcol):
    """dst = un-normalized RY on qubit q of src (both [128, SFREE] fp16).

    u0 = s0 - t*s1 ; u1 = t*s0 + s1  (cos factor deferred to the
    per-layer clp rescale). Two fused STT ops (no DVE fast mode, but
    the only 2-op form; used on Pool)."""
    co = 1 << (q + 1)
    inner = 1 << (9 - q)
    sv = src[:].rearrange("p (co t i) -> p co t i", co=co, t=2, i=inner)
    dv = dst[:].rearrange("p (co t i) -> p co t i", co=co, t=2, i=inner)
    s0 = sv[:, :, 0, :]
    s1 = sv[:, :, 1, :]
    eng.scalar_tensor_tensor(dv[:, :, 0, :], s1, ntcol, s0, ALU.mult, ALU.add)
    eng.scalar_tensor_tensor(dv[:, :, 1, :], s0, tcol, s1, ALU.mult, ALU.add)


def _emit_butterfly_ts(nc, src, dst, tm, q, tcol, prod_eng=None, sum_eng=None):
    """Butterfly via full-tile product + two half adds:
    tm = t*src (TS, DVE 4x fp16); u0 = s0 - tm1; u1 = s1 + tm0 (TT, 2x).
    """
    if prod_eng is None:
        prod_eng = nc.vector
    if sum_eng is None:
        sum_eng = nc.vector
    co = 1 << (q + 1)
    inner = 1 << (9 - q)
    if prod_eng is nc.scalar:
        prod_eng.activation(tm[:], src[:], AF.Identity, scale=tcol)
    else:
        prod_eng.tensor_scalar_mul(tm[:], src[:], tcol)
    sv = src[:].rearrange("p (co t i) -> p co t i", co=co, t=2, i=inner)
    dv = dst[:].rearrange("p (co t i) -> p co t i", co=co, t=2, i=inner)
    mv = tm[:].rearrange("p (co t i) -> p co t i", co=co, t=2, i=inner)
    sum_eng.tensor_tensor(dv[:, :, 0, :], sv[:, :, 0, :], mv[:, :, 1, :],
                          ALU.subtract)
    sum_eng.tensor_tensor(dv[:, :, 1, :], sv[:, :, 1, :], mv[:, :, 0, :],
                          ALU.add)


def _build_nc(rep=1):
    nc = bacc.Bacc()
    x_d = nc.declare_dram_parameter("x", [PER_CORE, OBS], F32, isOutput=False)
    tbl_d = nc.declare_dram_parameter("tbl", [TBL_LEN], F32, isOutput=False)
    wm_d = nc.declare_dram_parameter("wm", [NL, NAMP, NAMP], F16, isOutput=False)
    sgn_d = nc.declare_dram_parameter("sgn", [P, NAMP], F16, isOutput=False)
    out_d = nc.declare_dram_parameter("out", [PER_CORE, NOUT], F32, isOutput=True)

    with TileContext(nc) as tc:
        with tc.tile_pool(name="pool", bufs=1) as pool, \
             tc.tile_pool(name="psum", bufs=1, space="PSUM") as psum:
            # shared constants
            tbl = pool.tile([P, TBL_LEN], F32, tag="tbl")
            nc.sync.dma_start(
                out=tbl[:], in_=tbl_d[:].unsqueeze(0).to_broadcast((P, TBL_LEN))
            )
            sgn = pool.tile([P, NAMP], F16, tag="sgn")
            nc.sync.dma_start(out=sgn[:], in_=sgn_d[:])
            c_magic = pool.tile([P, 1], F32, tag="c_magic")
            c_halfpi = pool.tile([P, 1], F32, tag="c_halfpi")

            nc.vector.memset(c_magic[:], MAGIC)
            nc.vector.memset(c_halfpi[:], HALF_PI)

            stb2 = [pool.tile([P, SFREE], F16, tag=f"stb{i}", name=f"stb{i}")
                    for i in range(NCHAIN)]

            # ---- per-chain setup: x DMA + angle tables ----
            C = [dict() for _ in range(NCHAIN)]
            for ch in range(NCHAIN):
                tg = f"_{ch}"
                d = C[ch]
                xt = pool.tile([P, OBS], F32, tag="xt" + tg)
                nc.sync.dma_start(out=xt[:], in_=x_d[ch * P : (ch + 1) * P, :])

                ty = pool.tile([P, 5 * OBS], F32, tag="ty" + tg)
                sy = pool.tile([P, 5 * OBS], F32, tag="sy" + tg)
                cy = pool.tile([P, 5 * OBS], F32, tag="cy" + tg)
                rcy = pool.tile([P, 5 * OBS], F32, tag="rcy" + tg)
                nty = pool.tile([P, 5 * OBS], F32, tag="nty" + tg)
                php = pool.tile([P, 384], F32, tag="php" + tg)
                phl = pool.tile([P, 384], F32, tag="phl" + tg)
                ph16h = pool.tile([P, 384], F16, tag="ph16h" + tg)
                ph16l = pool.tile([P, 384], F16, tag="ph16l" + tg)
                phTh = pool.tile([P, 384], F16, tag="phTh" + tg)
                phTl = pool.tile([P, 384], F16, tag="phTl" + tg)
                clp = pool.tile([P, NL], F32, tag="clp" + tg)

                xb = xt[:].unsqueeze(1).to_broadcast((P, NL, OBS))
                tyv = ty[:].rearrange("p (l q) -> p l q", l=NL)
                ysv = tbl[:, O_YS : O_YS + 50].rearrange("p (l q) -> p l q", l=NL)
                nc.vector.tensor_tensor(tyv, xb, ysv, ALU.mult)
                nc.scalar.activation(sy[:], ty[:], AF.Sin)
                nc.vector.add_range_wrap(cy[:], ty[:], HALF_PI, PI, TWO_PI)
                nc.scalar.activation(cy[:], cy[:], AF.Sin)
                nc.vector.reciprocal(rcy[:], cy[:])
                nc.vector.tensor_tensor(ty[:], sy[:], rcy[:], ALU.mult)
                nc.vector.tensor_scalar_mul(nty[:], ty[:], -1.0)
                for l in range(NL):
                    nc.vector.tensor_reduce(
                        clp[:, l : l + 1], cy[:, 10 * l : 10 * l + 10],
                        mybir.AxisListType.X, ALU.mult,
                    )
                # phases in turns, fp16 hi/lo split, transposed for PE
                nc.vector.memset(php[:], 0.0)
                phv = php[:].rearrange("p (l v) -> p l v", l=6)[:, 0:5, 0:10]
                zsv = tbl[:, O_ZS : O_ZS + 50].rearrange("p (l q) -> p l q", l=NL)
                zwv = tbl[:, O_ZW : O_ZW + 50].rearrange("p (l q) -> p l q", l=NL)
                nc.vector.tensor_tensor(phv, xb, zsv, ALU.mult)
                nc.vector.tensor_tensor(phv, phv, zwv, ALU.add)
                nc.scalar.copy(ph16h[:], php[:])
                nc.scalar.copy(phl[:], ph16h[:])
                nc.vector.tensor_tensor(phl[:], php[:], phl[:], ALU.subtract)
                nc.scalar.copy(ph16l[:], phl[:])
                nc.sync.dma_start(
                    out=phTh[:].rearrange("p (r s) -> p r s", r=3),
                    in_=ph16h[:], transpose=True)
                nc.sync.dma_start(
                    out=phTl[:].rearrange("p (r s) -> p r s", r=3),
                    in_=ph16l[:], transpose=True)

                d["ty"], d["nty"] = ty, nty
                d["phTh"], d["phTl"] = phTh, phTl
                d["clp"] = clp
                d["sa"] = pool.tile([P, SFREE], F16, tag="sa" + tg, name="sa" + tg)
                d["sb"] = pool.tile([P, SFREE], F16, tag="sb" + tg, name="sb" + tg)
                d["cosT"] = pool.tile([P, NAMP], F16, tag="cosT" + tg, name="cosT" + tg)
                d["sinT"] = pool.tile([P, NAMP], F16, tag="sinT" + tg, name="sinT" + tg)
                d["tm"] = pool.tile([P, SFREE], F16, tag="tm" + tg, name="tm" + tg)
                d["tA"] = pool.tile([P, NAMP], F16, tag="tA" + tg, name="tA" + tg)
                d["tB"] = pool.tile([P, NAMP], F16, tag="tB" + tg, name="tB" + tg)
                d["ka"] = pool.tile([P, NAMP], F32, tag="ka" + tg, name="ka" + tg)
                d["nred"] = pool.tile([P, NAMP], F32, tag="nred" + tg, name="nred" + tg)
                d["mask"] = pool.tile([P, NAMP], F32, tag="mask" + tg, name="mask" + tg)
                d["cur"], d["oth"] = d["sa"], d["sb"]

            def col(t, l, i):
                return t[:, 10 * l + i : 10 * l + i + 1]

            # ---- circuit, layer-major (W tile shared across chains) ----
            from contextlib import nullcontext
            loop_cm = tc.For_i(0, rep, 1) if rep > 1 else nullcontext()
            with loop_cm:
                for l in range(NL):
                    # load this layer's 1024x1024 fp16 weight matrix (lhsT blocks)
                    wta = pool.tile([P, 4 * NAMP], F16, tag="wt_a", bufs=2,
                                    name=f"wta{l}")
                    wtb = pool.tile([P, 4 * NAMP], F16, tag="wt_b", bufs=2,
                                    name=f"wtb{l}")
                    nc.sync.dma_start(
                        out=wta[:].rearrange("p (r m) -> p r m", r=4),
                        in_=wm_d[l, 0 : 4 * P].rearrange("(r p) m -> p r m", p=P),
                    )
                    nc.sync.dma_start(
                        out=wtb[:].rearrange("p (r m) -> p r m", r=4),
                        in_=wm_d[l, 4 * P : 8 * P].rearrange("(r p) m -> p r m", p=P),
                    )
                    for ch in range(NCHAIN):
                        d = C[ch]
                        stb = stb2[ch]
                        cur, oth = d["cur"], d["oth"]
                        ty, nty = d["ty"], d["nty"]
                        cosT, sinT = d["cosT"], d["sinT"]
                        tA, tB = d["tA"], d["tB"]
                        ka, nred, mask = d["ka"], d["nred"], d["mask"]

                        # ---- phase table via PE matmul vs +-1 sign matrix ----
                        pht = psum.tile([P, NAMP], F32, tag="ps_ph", bufs=1,
                                        name="pht")
                        pb = 64 * (l % 2)
                        fb = (l // 2) * P
                        lhh = d["phTh"][pb : pb + 10, fb : fb + P]
                        lhl = d["phTl"][pb : pb + 10, fb : fb + P]
                        sgb = sgn[pb : pb + 10, :]
                        for li, lh in enumerate((lhh, lhl)):
                            for h in range(2):
                                nc.tensor.matmul(
                                    pht[:, h * 512 : (h + 1) * 512], lh,
                                    sgb[:, h * 512 : (h + 1) * 512],
                                    start=(li == 0), stop=(li == 1),
                                )
                        # magic round + (k - pht) = -red, all in turns
                        nc.scalar.activation(ka[:], pht[:], AF.Identity,
                                             bias=c_magic[:])
                        nc.vector.scalar_tensor_tensor(
                            nred[:], ka[:], MAGIC, pht[:], ALU.subtract,
                            ALU.subtract,
                        )
                        nc.scalar.activation(sinT[:], nred[:], AF.Sin,
                                             scale=-TWO_PI)
                        # cos(2*pi*red) = sin(pi/2 - 2*pi*|red|), always in range
                        nc.scalar.activation(mask[:], nred[:], AF.Abs)
                        nc.scalar.activation(cosT[:], mask[:], AF.Sin,
                                             scale=-TWO_PI, bias=c_halfpi[:])

                    for ch in range(NCHAIN):
                        d = C[ch]
                        stb = stb2[ch]
                        cur, oth = d["cur"], d["oth"]
                        ty, nty = d["ty"], d["nty"]
                        cosT, sinT = d["cosT"], d["sinT"]
                        tA, tB = d["tA"], d["tB"]

                        # ---- per-sample RY layer ----
                        if l == 0:
                            nc.vector.memset(cur[:, 0:1], 1.0)
                            for j in range(9, -1, -1):
                                g = 1 << (9 - j)
                                nc.vector.tensor_scalar_mul(
                                    cur[:, g : 2 * g], cur[:, 0:g], col(ty, 0, j)
                                )
                        else:
                            tm = d["tm"]
                            for i in range(9):
                                # a couple of products on Act to offload DVE
                                pe = nc.scalar if i in (0, 2, 4, 6, 8) else nc.vector
                                _emit_butterfly_ts(
                                    nc, cur, oth, tm, i, col(ty, l, i),
                                    prod_eng=pe,
                                )
                                cur, oth = oth, cur
                            _emit_butterfly_stt(
                                nc.vector, cur, oth, 9,
                                col(ty, l, 9), col(nty, l, 9)
                            )
                            cur, oth = oth, cur

                        # ---- diag multiply by exp(i*phase) ----
                        sre = cur[:, 0:NAMP]
                        sim = cur[:, NAMP:SFREE]
                        dre = oth[:, 0:NAMP]
                        dim = oth[:, NAMP:SFREE]
                        if l == 0:
                            nc.vector.tensor_tensor(dre, sre, cosT[:], ALU.mult)
                            nc.vector.tensor_tensor(dim, sre, sinT[:], ALU.mult)
                        else:
                            nc.vector.tensor_tensor(tA[:], sim, sinT[:], ALU.mult)
                            nc.vector.tensor_tensor(tB[:], sim, cosT[:], ALU.mult)
                            nc.vector.tensor_tensor(dre, sre, cosT[:], ALU.mult)
                            nc.vector.tensor_tensor(dim, sre, sinT[:], ALU.mult)
                            nc.vector.tensor_tensor(dre, dre, tA[:], ALU.subtract)
                            nc.vector.tensor_tensor(dim, dim, tB[:], ALU.add)
                        cur, oth = oth, cur

                        # ---- shared RY + CNOT ring as one matmul on PE ----
                        # DMA-XBAR transpose: stb[p, r, s] = cur[s, r*128+p]
                        for comp in range(2):
                            nc.sync.dma_start(
                                out=stb[:, comp * NAMP : (comp + 1) * NAMP]
                                    .rearrange("p (r s) -> p r s", r=8),
                                in_=cur[:, comp * NAMP : (comp + 1) * NAMP],
                                transpose=True,
                            )
                        for comp in range(2):
                            pm = psum.tile([P, NAMP], F32, tag="ps_mm", bufs=3,
                                           name="pm")
                            for r in range(8):
                                lhsT = stb[:, comp * NAMP + r * P :
                                           comp * NAMP + (r + 1) * P]
                                wh = wta if r < 4 else wtb
                                rr = r % 4
                                for h in range(2):
                                    nc.tensor.matmul(
                                        pm[:, h * 512 : (h + 1) * 512], lhsT,
                                        wh[:, rr * NAMP + h * 512 :
                                           rr * NAMP + (h + 1) * 512],
                                        start=(r == 0), stop=(r == 7),
                                    )
                            nc.scalar.activation(
                                oth[:, comp * NAMP : (comp + 1) * NAMP],
                                pm[:], AF.Identity,
                                scale=d["clp"][:, l : l + 1],
                            )
                        cur, oth = oth, cur
                        d["cur"], d["oth"] = cur, oth

            # ---- observables ----
            for ch in range(NCHAIN):
                tg = f"_{ch}"
                d = C[ch]
                cur = d["cur"]
                pr, pi = d["ka"], d["nred"]
                zt = pool.tile([P, 8], F32, tag="zt" + tg)
                ot = pool.tile([P, NOUT], F32, tag="ot" + tg)
                sre = cur[:, 0:NAMP]
                sim = cur[:, NAMP:SFREE]
                nc.scalar.activation(pr[:], sre, AF.Square)
                nc.scalar.activation(pi[:], sim, AF.Square)
                nc.vector.tensor_tensor(pr[:], pr[:], pi[:], ALU.add)
                nc.vector.tensor_reduce(
                    zt[:, 4:5], pr[:], mybir.AxisListType.X, ALU.add
                )
                for i in range(NOUT):
                    o = 1 << i
                    inner = 1 << (9 - i)
                    pv = pr[:].rearrange("p (o t i) -> p o t i", o=o, t=2, i=inner)
                    nc.vector.tensor_reduce(
                        zt[:, i : i + 1], pv[:, :, 0, :],
                        mybir.AxisListType.XY, ALU.add,
                    )
                for i in range(NOUT):
                    nc.vector.tensor_scalar(
                        ot[:, i : i + 1], zt[:, i : i + 1], 2.0, zt[:, 4:5],
                        ALU.mult, ALU.subtract,
                    )
                nc.vector.tensor_tensor(
                    ot[:], ot[:], tbl[:, O_OS : O_OS + NOUT], ALU.mult
                )
                nc.sync.dma_start(
                    out=out_d[ch * P : (ch + 1) * P, :], in_=ot[:]
                )

    nc.compile()
    return nc


def _host_table(input_scaling, weights, output_scaling):
    isc = np.asarray(input_scaling, np.float64)
    w = np.asarray(weights, np.float64)
    os_ = np.asarray(output_scaling, np.float64)
    ys = 0.5 * isc[:, :OBS]
    zs = 0.5 * isc[:, OBS : 2 * OBS] / (2 * np.pi)
    zw = 0.5 * w[:, :NQ] / (2 * np.pi)
    tbl = np.concatenate(
        [ys.ravel(), zs.ravel(), zw.ravel(), os_]
    ).astype(np.float32)
    assert tbl.shape[0] == TBL_LEN
    return tbl


def _host_mats(weights):
    """Per-layer 1024x1024 lhsT matrices: W_l = (P_ring @ kron_i RY(w2_i)).T"""
    w = np.asarray(weights, np.float64)
    # CNOT-ring permutation L: bits b0(MSB)..b9; b_{i+1}^=b_i (i=0..8), b0^=b9
    s = np.arange(NAMP)
    bits = [(s >> (9 - j)) & 1 for j in range(10)]
    for i in range(9):
        bits[i + 1] = bits[i + 1] ^ bits[i]
    bits[0] = bits[0] ^ bits[9]
    L = np.zeros(NAMP, np.int64)
    for j in range(10):
        L |= bits[j] << (9 - j)
    wm = np.empty((NL, NAMP, NAMP), np.float16)
    for l in range(NL):
        M = np.array([[1.0]])
        for i in range(NQ):
            a = 0.5 * w[l, NQ + i]
            c, sn = np.cos(a), np.sin(a)
            M = np.kron(M, np.array([[c, -sn], [sn, c]]))
        Ml = np.zeros_like(M)
        Ml[L, :] = M          # ring permutation applied after the rotations
        wm[l] = Ml.T.astype(np.float16)
    return wm


def _host_sgn():
    amp = np.arange(NAMP)
    sgn = np.zeros((P, NAMP), np.float16)
    for j in range(NQ):
        row = np.where(((amp >> (9 - j)) & 1) == 1, 1.0, -1.0)
        sgn[j] = row
        sgn[64 + j] = row
    return sgn


def kernel(x, input_scaling, weights, output_scaling):
    global _CACHED_NC
    x = np.ascontiguousarray(np.asarray(x, np.float32))
    tbl = _host_table(input_scaling, weights, output_scaling)
    wm = _host_mats(weights)
    sgn = _host_sgn()

    if 1 not in _CACHED_NC:
        _CACHED_NC[1] = _build_nc(1)
    nc = _CACHED_NC[1]

    in_maps = [
        {"x": x[c * PER_CORE : (c + 1) * PER_CORE], "tbl": tbl,
         "wm": wm, "sgn": sgn}
        for c in range(N_CORES)
    ]
    res = run_bass_kernel_spmd(nc, in_maps, list(range(N_CORES))).results
    return np.concatenate([r["out"] for r in res], axis=0)


if __name__ == "__main__":
    rng = np.random.default_rng(0)
    x = rng.standard_normal((BATCH, OBS)).astype(np.float32)
    isc = np.ones((NL, 2 * NQ), np.float32)
    w = rng.uniform(-np.pi, np.pi, (NL, 2 * NQ)).astype(np.float32)
    os_ = np.ones((NOUT,), np.float32)
    out = kernel(x, isc, w, os_)
    print(out.shape, out[:2])
